# revision 9
# baseline (speedup 1.0000x reference)
"""Trainium2 Bass kernel for nn_CenterCrop: per-sample resize(short-side=256)
+ center-crop(224), bilinear, batch sharded over 8 NeuronCores.

Bilinear resize is separable: out = S^T @ img @ G with per-sample sparse
interpolation matrices S (vertical) and G (horizontal), built on the host
from the h/w metadata. The gather+lerp runs on the PE array as bf16 matmuls
with fp32 PSUM accumulation (rel err ~7e-3 vs the 2e-2 gate):
  pass1: tmp1_T[x, j] = sum_y img[y, x] * S[y, j]   (img tiles stationary)
  pass2: out[j, i]    = sum_x tmp1_T[x, j] * G[x, i] (tmp1 tiles stationary)

Perf structure:
- Only the per-sample source window that the output reads (~0.875*min(h,w)
  squared) is DMA'd/processed; S/G are banded and only bands are packed,
  DMA'd, and streamed (PSUM has_written bits make split accumulation exact).
- Inputs per slot are packed into TWO [128, *] bf16 tensors laid out
  partition-major (one linear DMA each): A = S bands + channel-0 window,
  B = G bands + channel-1/2 windows. pass1 c0 only waits on A, shrinking
  the initial fill; A frees after pass1.
- PSUM->SBUF drains run on the Scalar engine (ACT Copy, 172+FD cyc, idle
  otherwise); the Vector engine only does the pass1 tmp casts. Output is
  bf16 [112, 2, 224] per (slot, channel), DMA'd per channel, host unpermutes
  and upcasts.
- SPMD requires one program for all 8 cores, so samples are sorted by
  min(h,w) and dealt round-robin: slot s on every core holds same-sized
  windows; the program is specialized per-slot to the union shape/bands.
  Slot order small-first (fast fill) and small-last (fast tail).

History (HW, 8 cores): fp32 exact 117.8us -> bf16 single-pass 67.4us ->
packed single-DMA inputs + bf16 out 60.1us -> this version.
"""

import sys
import os

for _p in ("/opt/trn_rl_repo",):
    if os.path.isdir(_p) and _p not in sys.path:
        sys.path.insert(0, _p)

import numpy as np
import ml_dtypes

BF16 = ml_dtypes.bfloat16

OUT_H = 224
OUT_W = 224
RESIZE_TO = np.float32(256.0)
B_FULL = 64
N_CORES = 8
B_LOC = B_FULL // N_CORES  # 8 slots per core
C = 3
H = 512
W = 512  # image width after stripping the metadata column (stored width 513)

LAST_EXEC_NS = None
LAST_RESULTS = None
_NC_CACHE = {}


def _interp_matrices(h, w):
    """Full S [512, OUT_H], G [512, OUT_W] fp32 interpolation matrices,
    mirroring the reference fp32 math bit-for-bit."""
    f32 = np.float32
    h = f32(h)
    w = f32(w)
    min_dim = min(h, w)
    scale = RESIZE_TO / min_dim
    h_res = np.round(h * scale)
    w_res = np.round(w * scale)
    top = np.round((h_res - f32(OUT_H)) / f32(2.0))
    left = np.round((w_res - f32(OUT_W)) / f32(2.0))

    def axis_mat(n_out, offset, dim, dim_res, n_src):
        idx = np.arange(n_out, dtype=np.float32) + offset
        src = np.clip((idx + f32(0.5)) * dim / dim_res - f32(0.5),
                      f32(0.0), dim - f32(1.0))
        p0f = np.floor(src)
        frac = src - p0f
        imax = np.int32(dim) - 1
        p0 = np.clip(p0f.astype(np.int32), 0, imax)
        p1 = np.minimum(p0 + 1, imax)
        mat = np.zeros((n_src, n_out), np.float32)
        cols = np.arange(n_out)
        np.add.at(mat, (p0, cols), f32(1.0) - frac)
        np.add.at(mat, (p1, cols), frac)
        return mat

    S = axis_mat(OUT_H, top, h, h_res, H)
    G = axis_mat(OUT_W, left, w, w_res, W)
    return S, G


def _bands(mat_w, n_tiles):
    """Per-128-row-tile [lo, hi) columns with any nonzero; None if empty."""
    out = []
    for t in range(n_tiles):
        rows = mat_w[t * 128:(t + 1) * 128]
        nz = np.nonzero(rows.any(axis=0))[0]
        out.append(None if nz.size == 0 else (int(nz[0]), int(nz[-1]) + 1))
    return out


def _union_bands(band_lists):
    n = len(band_lists[0])
    out = []
    for t in range(n):
        los = [b[t][0] for b in band_lists if b[t] is not None]
        his = [b[t][1] for b in band_lists if b[t] is not None]
        out.append(None if not los else (min(los), max(his)))
    return out


def _offsets(bands):
    """Packed offsets for the non-empty bands; returns (offs, total)."""
    offs = []
    tot = 0
    for b in bands:
        if b is None:
            offs.append(None)
        else:
            offs.append(tot)
            tot += b[1] - b[0]
    return tuple(offs), tot


def _prepare(x):
    """Host prep: per-sample windows/matrices, sorted slot assignment,
    per-core packed A/B tensors, and per-slot program params."""
    h_all = x[:, 0, 0, -1].astype(np.float32)
    w_all = x[:, 1, 0, -1].astype(np.float32)

    samples = []
    for b in range(B_FULL):
        S, G = _interp_matrices(h_all[b], w_all[b])
        ynz = np.nonzero(S.any(axis=1))[0]
        xnz = np.nonzero(G.any(axis=1))[0]
        y0, y1 = int(ynz[0]), int(ynz[-1]) + 1
        x0, x1 = int(xnz[0]), int(xnz[-1]) + 1
        samples.append(dict(S=S[y0:y1], G=G[x0:x1], y0=y0, x0=x0,
                            wh=y1 - y0, ww=x1 - x0))

    order = np.argsort(np.minimum(h_all, w_all), kind="stable")
    # slot s, core c -> sample order[s*N_CORES + c]
    assign = [[int(order[s * N_CORES + c]) for c in range(N_CORES)]
              for s in range(B_LOC)]

    slot_params = []
    in_maps = [{} for _ in range(N_CORES)]
    for s in range(B_LOC):
        sids = assign[s]
        wh = max(samples[i]["wh"] for i in sids)
        ww = max(samples[i]["ww"] for i in sids)
        n_yt = (wh + 127) // 128
        n_xt = (ww + 127) // 128
        sb_list, gb_list = [], []
        for i in sids:
            sp = samples[i]
            Sw = np.zeros((n_yt * 128, OUT_H), np.float32)
            Sw[:sp["wh"]] = sp["S"]
            Gw = np.zeros((n_xt * 128, OUT_W), np.float32)
            Gw[:sp["ww"]] = sp["G"]
            sb_list.append(_bands(Sw, n_yt))
            gb_list.append(_bands(Gw, n_xt))
        sbands = tuple(_union_bands(sb_list))
        gbands = tuple(_union_bands(gb_list))
        s_offs, s_tot = _offsets(sbands)
        g_offs, g_tot = _offsets(gbands)
        cw = n_yt * ww  # one channel's window elems per partition
        tot_a = s_tot + cw
        tot_b = g_tot + 2 * cw
        slot_params.append((n_yt, n_xt, ww, sbands, gbands,
                            s_offs, g_offs, s_tot, g_tot, tot_a, tot_b))

        for c_core in range(N_CORES):
            sid = sids[c_core]
            sp = samples[sid]
            Sw = np.zeros((n_yt * 128, OUT_H), np.float32)
            Sw[:sp["wh"]] = sp["S"]
            Gw = np.zeros((n_xt * 128, OUT_W), np.float32)
            Gw[:sp["ww"]] = sp["G"]
            arr_a = np.zeros((128, tot_a), BF16)
            arr_b = np.zeros((128, tot_b), BF16)
            for t in range(n_yt):
                if sbands[t] is None:
                    continue
                lo, hi = sbands[t]
                off = s_offs[t]
                arr_a[:, off:off + hi - lo] = Sw[t * 128:(t + 1) * 128, lo:hi]
            for t in range(n_xt):
                if gbands[t] is None:
                    continue
                lo, hi = gbands[t]
                off = g_offs[t]
                arr_b[:, off:off + hi - lo] = Gw[t * 128:(t + 1) * 128, lo:hi]
            # image windows, partition-major: row t*128+p -> [p, t*ww + x]
            xw = np.zeros((C, n_yt * 128, ww), np.float32)
            xw[:, :sp["wh"], :sp["ww"]] = x[
                sid, :, sp["y0"]:sp["y0"] + sp["wh"],
                sp["x0"]:sp["x0"] + sp["ww"]]
            xw_t = xw.reshape(C, n_yt, 128, ww).transpose(2, 0, 1, 3)
            arr_a[:, s_tot:] = xw_t[:, 0].reshape(128, cw)
            arr_b[:, g_tot:] = xw_t[:, 1:].reshape(128, 2 * cw)
            in_maps[c_core][f"ina{s}"] = arr_a
            in_maps[c_core][f"inb{s}"] = arr_b
    return tuple(slot_params), in_maps, assign


def _build_nc(slot_params):
    import concourse.bacc as bacc
    import concourse.mybir as mybir
    import concourse.tile as tile

    dt = mybir.dt.float32
    dtb = mybir.dt.bfloat16
    act_copy = mybir.ActivationFunctionType.Copy
    nc = bacc.Bacc(
        "TRN2",
        target_bir_lowering=False,
        debug=False,
        enable_asserts=False,
        num_devices=N_CORES,
    )
    in_a = [nc.dram_tensor(f"ina{s}", [128, p[-2]], dtb, kind="ExternalInput")
            for s, p in enumerate(slot_params)]
    in_b = [nc.dram_tensor(f"inb{s}", [128, p[-1]], dtb, kind="ExternalInput")
            for s, p in enumerate(slot_params)]
    out = nc.dram_tensor("out", [B_LOC, 112, C, 2, OUT_W], dtb,
                         kind="ExternalOutput")

    # small first (fast fill), 2nd-smallest last (fast drain)
    slot_order = [0] + list(range(2, B_LOC)) + [1]

    with tile.TileContext(nc) as tc:
        with (
            tc.tile_pool(name="ina", bufs=2) as a_pool,
            tc.tile_pool(name="inb", bufs=2) as b_pool,
            tc.tile_pool(name="tmp", bufs=4) as tmp_pool,
            tc.tile_pool(name="outp", bufs=2) as out_pool,
            tc.tile_pool(name="ps1", bufs=3, space="PSUM") as ps1_pool,
            tc.tile_pool(name="ps2", bufs=3, space="PSUM") as ps2_pool,
        ):
            for s in slot_order:
                (n_yt, n_xt, ww, sbands, gbands, s_offs, g_offs,
                 s_tot, g_tot, tot_a, tot_b) = slot_params[s]
                a_sb = a_pool.tile([128, tot_a], dtb)
                nc.sync.dma_start(a_sb[:], in_a[s][:])
                b_sb = b_pool.tile([128, tot_b], dtb)
                nc.sync.dma_start(b_sb[:], in_b[s][:])
                cw = n_yt * ww
                s_emit = [t for t in range(n_yt) if sbands[t] is not None]
                g_emit = [t for t in range(n_xt) if gbands[t] is not None]
                # pass1 for all channels first: pass2's wait on the last tmp
                # cast then overlaps other channels' pass1 matmuls instead of
                # head-of-line-blocking the PE queue.
                tmps = []
                for c in range(C):
                    tmp_sb = tmp_pool.tile([128, n_xt, OUT_H], dtb)
                    tmps.append(tmp_sb)
                    for xb in range(n_xt):
                        xlo = xb * 128
                        xn = min(128, ww - xlo)
                        ps1 = ps1_pool.tile([128, OUT_H], dt)
                        for i_t, t in enumerate(s_emit):
                            lo, hi = sbands[t]
                            if c == 0:
                                img = a_sb[:, s_tot + t * ww + xlo:
                                           s_tot + t * ww + xlo + xn]
                            else:
                                ib = g_tot + ((c - 1) * n_yt + t) * ww + xlo
                                img = b_sb[:, ib:ib + xn]
                            so = s_offs[t]
                            nc.tensor.matmul(
                                ps1[:xn, lo:hi],
                                img,
                                a_sb[:, so:so + hi - lo],
                                start=(i_t == 0),
                                stop=(i_t == len(s_emit) - 1),
                                skip_group_check=True,
                            )
                        nc.vector.tensor_copy(tmp_sb[:xn, xb, :],
                                              ps1[:xn, :OUT_H])
                out_sb = out_pool.tile([112, C, 2, OUT_W], dtb)
                for c in range(C):
                    ps2 = ps2_pool.tile([112, 2, OUT_W], dt)
                    for jb in range(2):
                        for i_t, xb in enumerate(g_emit):
                            lo, hi = gbands[xb]
                            xn = min(128, ww - xb * 128)
                            go = g_offs[xb]
                            nc.tensor.matmul(
                                ps2[:, jb, lo:hi],
                                tmps[c][:xn, xb, jb * 112:(jb + 1) * 112],
                                b_sb[:xn, go:go + hi - lo],
                                start=(i_t == 0),
                                stop=(i_t == len(g_emit) - 1),
                                skip_group_check=True,
                            )
                    # split the drain: ScE takes jb=0, DVE takes jb=1
                    nc.scalar.activation(out_sb[:, c, 0, :], ps2[:, 0, :],
                                         act_copy)
                    nc.vector.tensor_copy(out_sb[:, c, 1, :], ps2[:, 1, :])
                nc.sync.dma_start(out[s], out_sb[:])
    nc.compile()
    return nc


def kernel(x, _trace=False):
    global LAST_EXEC_NS, LAST_RESULTS
    from concourse.bass_utils import run_bass_kernel_spmd

    x = np.ascontiguousarray(np.asarray(x), dtype=np.float32)
    assert x.shape == (B_FULL, C, H, W + 1), x.shape

    slot_params, in_maps, assign = _prepare(x)
    key = slot_params
    if key not in _NC_CACHE:
        _NC_CACHE[key] = _build_nc(slot_params)
    nc = _NC_CACHE[key]

    res = run_bass_kernel_spmd(nc, in_maps, list(range(N_CORES)), trace=_trace)
    LAST_EXEC_NS = res.exec_time_ns
    LAST_RESULTS = res

    out_full = np.empty((B_FULL, C, OUT_H, OUT_W), np.float32)
    for s in range(B_LOC):
        for c in range(N_CORES):
            # [112, C, 2, 224] -> [C, 2, 112, 224] -> [C, 224, 224]
            arr = np.asarray(res.results[c]["out"][s]).astype(np.float32)
            out_full[assign[s][c]] = arr.transpose(1, 2, 0, 3).reshape(
                C, OUT_H, OUT_W)
    return out_full


# revision 11
# speedup vs baseline: 1.0266x; 1.0266x over previous
"""Trainium2 Bass kernel for nn_CenterCrop: per-sample resize(short-side=256)
+ center-crop(224), bilinear, batch sharded over 8 NeuronCores.

Bilinear resize is separable: out = S^T @ img @ G with per-sample sparse
interpolation matrices S (vertical) and G (horizontal), built on the host
from the h/w metadata. The gather+lerp runs on the PE array as bf16 matmuls
with fp32 PSUM accumulation (rel err ~7e-3 vs the 2e-2 gate):
  pass1: tmp1_T[x, j] = sum_y img[y, x] * S[y, j]   (img tiles stationary)
  pass2: out[j, i]    = sum_x tmp1_T[x, j] * G[x, i] (tmp1 tiles stationary)

Perf structure:
- Only the per-sample source window that the output reads (~0.875*min(h,w)
  squared) is DMA'd/processed; S/G are banded and only bands are packed,
  DMA'd, and streamed (PSUM has_written bits make split accumulation exact).
- Inputs per slot are packed into TWO [128, *] bf16 tensors laid out
  partition-major (one linear DMA each): A = S bands + channel-0 window,
  B = G bands + channel-1/2 windows. pass1 c0 only waits on A, shrinking
  the initial fill; A frees after pass1.
- PSUM->SBUF drains run on the Scalar engine (ACT Copy, 172+FD cyc, idle
  otherwise); the Vector engine only does the pass1 tmp casts. Output is
  bf16 [112, 2, 224] per (slot, channel), DMA'd per channel, host unpermutes
  and upcasts.
- SPMD requires one program for all 8 cores, so samples are sorted by
  min(h,w) and dealt round-robin: slot s on every core holds same-sized
  windows; the program is specialized per-slot to the union shape/bands.
  Slot order small-first (fast fill) and small-last (fast tail).

History (HW, 8 cores): fp32 exact 117.8us -> bf16 single-pass 67.4us ->
packed single-DMA inputs + bf16 out 60.1us -> this version.
"""

import sys
import os

for _p in ("/opt/trn_rl_repo",):
    if os.path.isdir(_p) and _p not in sys.path:
        sys.path.insert(0, _p)

import numpy as np
import ml_dtypes

BF16 = ml_dtypes.bfloat16

OUT_H = 224
OUT_W = 224
RESIZE_TO = np.float32(256.0)
B_FULL = 64
N_CORES = 8
B_LOC = B_FULL // N_CORES  # 8 slots per core
C = 3
H = 512
W = 512  # image width after stripping the metadata column (stored width 513)

LAST_EXEC_NS = None
LAST_RESULTS = None
_NC_CACHE = {}


def _interp_matrices(h, w):
    """Full S [512, OUT_H], G [512, OUT_W] fp32 interpolation matrices,
    mirroring the reference fp32 math bit-for-bit."""
    f32 = np.float32
    h = f32(h)
    w = f32(w)
    min_dim = min(h, w)
    scale = RESIZE_TO / min_dim
    h_res = np.round(h * scale)
    w_res = np.round(w * scale)
    top = np.round((h_res - f32(OUT_H)) / f32(2.0))
    left = np.round((w_res - f32(OUT_W)) / f32(2.0))

    def axis_mat(n_out, offset, dim, dim_res, n_src):
        idx = np.arange(n_out, dtype=np.float32) + offset
        src = np.clip((idx + f32(0.5)) * dim / dim_res - f32(0.5),
                      f32(0.0), dim - f32(1.0))
        p0f = np.floor(src)
        frac = src - p0f
        imax = np.int32(dim) - 1
        p0 = np.clip(p0f.astype(np.int32), 0, imax)
        p1 = np.minimum(p0 + 1, imax)
        mat = np.zeros((n_src, n_out), np.float32)
        cols = np.arange(n_out)
        np.add.at(mat, (p0, cols), f32(1.0) - frac)
        np.add.at(mat, (p1, cols), frac)
        return mat

    S = axis_mat(OUT_H, top, h, h_res, H)
    G = axis_mat(OUT_W, left, w, w_res, W)
    return S, G


def _bands(mat_w, n_tiles):
    """Per-128-row-tile [lo, hi) columns with any nonzero; None if empty."""
    out = []
    for t in range(n_tiles):
        rows = mat_w[t * 128:(t + 1) * 128]
        nz = np.nonzero(rows.any(axis=0))[0]
        out.append(None if nz.size == 0 else (int(nz[0]), int(nz[-1]) + 1))
    return out


def _union_bands(band_lists):
    n = len(band_lists[0])
    out = []
    for t in range(n):
        los = [b[t][0] for b in band_lists if b[t] is not None]
        his = [b[t][1] for b in band_lists if b[t] is not None]
        out.append(None if not los else (min(los), max(his)))
    return out


def _offsets(bands):
    """Packed offsets for the non-empty bands; returns (offs, total)."""
    offs = []
    tot = 0
    for b in bands:
        if b is None:
            offs.append(None)
        else:
            offs.append(tot)
            tot += b[1] - b[0]
    return tuple(offs), tot


def _prepare(x):
    """Host prep: per-sample windows/matrices, sorted slot assignment,
    per-core packed A/B tensors, and per-slot program params."""
    h_all = x[:, 0, 0, -1].astype(np.float32)
    w_all = x[:, 1, 0, -1].astype(np.float32)

    samples = []
    for b in range(B_FULL):
        S, G = _interp_matrices(h_all[b], w_all[b])
        ynz = np.nonzero(S.any(axis=1))[0]
        xnz = np.nonzero(G.any(axis=1))[0]
        y0, y1 = int(ynz[0]), int(ynz[-1]) + 1
        x0, x1 = int(xnz[0]), int(xnz[-1]) + 1
        samples.append(dict(S=S[y0:y1], G=G[x0:x1], y0=y0, x0=x0,
                            wh=y1 - y0, ww=x1 - x0))

    order = np.argsort(np.minimum(h_all, w_all), kind="stable")
    # slot s, core c -> sample order[s*N_CORES + c]
    assign = [[int(order[s * N_CORES + c]) for c in range(N_CORES)]
              for s in range(B_LOC)]

    slot_params = []
    in_maps = [{} for _ in range(N_CORES)]
    for s in range(B_LOC):
        sids = assign[s]
        wh = max(samples[i]["wh"] for i in sids)
        ww = max(samples[i]["ww"] for i in sids)
        n_yt = (wh + 127) // 128
        n_xt = (ww + 127) // 128
        sb_list, gb_list = [], []
        for i in sids:
            sp = samples[i]
            Sw = np.zeros((n_yt * 128, OUT_H), np.float32)
            Sw[:sp["wh"]] = sp["S"]
            Gw = np.zeros((n_xt * 128, OUT_W), np.float32)
            Gw[:sp["ww"]] = sp["G"]
            sb_list.append(_bands(Sw, n_yt))
            gb_list.append(_bands(Gw, n_xt))
        sbands = tuple(_union_bands(sb_list))
        gbands = tuple(_union_bands(gb_list))
        s_offs, s_tot = _offsets(sbands)
        g_offs, g_tot = _offsets(gbands)
        cw = n_yt * ww  # one channel's window elems per partition
        tot_a = s_tot + cw
        tot_b = g_tot + 2 * cw
        slot_params.append((n_yt, n_xt, ww, sbands, gbands,
                            s_offs, g_offs, s_tot, g_tot, tot_a, tot_b))

        for c_core in range(N_CORES):
            sid = sids[c_core]
            sp = samples[sid]
            Sw = np.zeros((n_yt * 128, OUT_H), np.float32)
            Sw[:sp["wh"]] = sp["S"]
            Gw = np.zeros((n_xt * 128, OUT_W), np.float32)
            Gw[:sp["ww"]] = sp["G"]
            arr_a = np.zeros((128, tot_a), BF16)
            arr_b = np.zeros((128, tot_b), BF16)
            for t in range(n_yt):
                if sbands[t] is None:
                    continue
                lo, hi = sbands[t]
                off = s_offs[t]
                arr_a[:, off:off + hi - lo] = Sw[t * 128:(t + 1) * 128, lo:hi]
            for t in range(n_xt):
                if gbands[t] is None:
                    continue
                lo, hi = gbands[t]
                off = g_offs[t]
                arr_b[:, off:off + hi - lo] = Gw[t * 128:(t + 1) * 128, lo:hi]
            # image windows, partition-major: row t*128+p -> [p, t*ww + x]
            xw = np.zeros((C, n_yt * 128, ww), np.float32)
            xw[:, :sp["wh"], :sp["ww"]] = x[
                sid, :, sp["y0"]:sp["y0"] + sp["wh"],
                sp["x0"]:sp["x0"] + sp["ww"]]
            xw_t = xw.reshape(C, n_yt, 128, ww).transpose(2, 0, 1, 3)
            arr_a[:, s_tot:] = xw_t[:, 0].reshape(128, cw)
            arr_b[:, g_tot:] = xw_t[:, 1:].reshape(128, 2 * cw)
            in_maps[c_core][f"ina{s}"] = arr_a
            in_maps[c_core][f"inb{s}"] = arr_b
    return tuple(slot_params), in_maps, assign


def _build_nc(slot_params):
    import concourse.bacc as bacc
    import concourse.mybir as mybir
    import concourse.tile as tile

    dt = mybir.dt.float32
    dtb = mybir.dt.bfloat16
    act_copy = mybir.ActivationFunctionType.Copy
    nc = bacc.Bacc(
        "TRN2",
        target_bir_lowering=False,
        debug=False,
        enable_asserts=False,
        num_devices=N_CORES,
    )
    in_a = [nc.dram_tensor(f"ina{s}", [128, p[-2]], dtb, kind="ExternalInput")
            for s, p in enumerate(slot_params)]
    in_b = [nc.dram_tensor(f"inb{s}", [128, p[-1]], dtb, kind="ExternalInput")
            for s, p in enumerate(slot_params)]
    out = nc.dram_tensor("out", [B_LOC, 112, C, 2, OUT_W], dtb,
                         kind="ExternalOutput")

    # small first (fast fill), 2nd-smallest last (fast drain)
    slot_order = [0] + list(range(2, B_LOC)) + [1]

    with tile.TileContext(nc) as tc:
        with (
            tc.tile_pool(name="ina", bufs=2) as a_pool,
            tc.tile_pool(name="inb", bufs=2) as b_pool,
            tc.tile_pool(name="tmp", bufs=4) as tmp_pool,
            tc.tile_pool(name="outp", bufs=2) as out_pool,
            tc.tile_pool(name="ps1", bufs=4, space="PSUM") as ps1_pool,
            tc.tile_pool(name="ps2", bufs=3, space="PSUM") as ps2_pool,
        ):
            for s in slot_order:
                (n_yt, n_xt, ww, sbands, gbands, s_offs, g_offs,
                 s_tot, g_tot, tot_a, tot_b) = slot_params[s]
                a_sb = a_pool.tile([128, tot_a], dtb)
                nc.sync.dma_start(a_sb[:], in_a[s][:])
                b_sb = b_pool.tile([128, tot_b], dtb)
                nc.sync.dma_start(b_sb[:], in_b[s][:])
                cw = n_yt * ww
                s_emit = [t for t in range(n_yt) if sbands[t] is not None]
                g_emit = [t for t in range(n_xt) if gbands[t] is not None]
                # pass1 for all channels first: pass2's wait on the last tmp
                # cast then overlaps other channels' pass1 matmuls instead of
                # head-of-line-blocking the PE queue.
                tmps = []
                for c in range(C):
                    tmp_sb = tmp_pool.tile([128, n_xt, OUT_H], dtb)
                    tmps.append(tmp_sb)
                    for xb in range(n_xt):
                        xlo = xb * 128
                        xn = min(128, ww - xlo)
                        ps1 = ps1_pool.tile([128, OUT_H], dt)
                        for i_t, t in enumerate(s_emit):
                            lo, hi = sbands[t]
                            if c == 0:
                                img = a_sb[:, s_tot + t * ww + xlo:
                                           s_tot + t * ww + xlo + xn]
                            else:
                                ib = g_tot + ((c - 1) * n_yt + t) * ww + xlo
                                img = b_sb[:, ib:ib + xn]
                            so = s_offs[t]
                            nc.tensor.matmul(
                                ps1[:xn, lo:hi],
                                img,
                                a_sb[:, so:so + hi - lo],
                                start=(i_t == 0),
                                stop=(i_t == len(s_emit) - 1),
                                skip_group_check=True,
                            )
                        # split PSUM drains across both copy engines
                        if (c * n_xt + xb) % 2 == 0:
                            nc.vector.tensor_copy(tmp_sb[:xn, xb, :],
                                                  ps1[:xn, :OUT_H])
                        else:
                            nc.scalar.activation(tmp_sb[:xn, xb, :],
                                                 ps1[:xn, :OUT_H], act_copy)
                out_sb = out_pool.tile([112, C, 2, OUT_W], dtb)
                for c in range(C):
                    ps2 = ps2_pool.tile([112, 2, OUT_W], dt)
                    for jb in range(2):
                        for i_t, xb in enumerate(g_emit):
                            lo, hi = gbands[xb]
                            xn = min(128, ww - xb * 128)
                            go = g_offs[xb]
                            nc.tensor.matmul(
                                ps2[:, jb, lo:hi],
                                tmps[c][:xn, xb, jb * 112:(jb + 1) * 112],
                                b_sb[:xn, go:go + hi - lo],
                                start=(i_t == 0),
                                stop=(i_t == len(g_emit) - 1),
                                skip_group_check=True,
                            )
                    # split the drain: ScE takes jb=0, DVE takes jb=1
                    nc.scalar.activation(out_sb[:, c, 0, :], ps2[:, 0, :],
                                         act_copy)
                    nc.vector.tensor_copy(out_sb[:, c, 1, :], ps2[:, 1, :])
                nc.sync.dma_start(out[s], out_sb[:])
    nc.compile()
    return nc


def kernel(x, _trace=False):
    global LAST_EXEC_NS, LAST_RESULTS
    from concourse.bass_utils import run_bass_kernel_spmd

    x = np.ascontiguousarray(np.asarray(x), dtype=np.float32)
    assert x.shape == (B_FULL, C, H, W + 1), x.shape

    slot_params, in_maps, assign = _prepare(x)
    key = slot_params
    if key not in _NC_CACHE:
        _NC_CACHE[key] = _build_nc(slot_params)
    nc = _NC_CACHE[key]

    res = run_bass_kernel_spmd(nc, in_maps, list(range(N_CORES)), trace=_trace)
    LAST_EXEC_NS = res.exec_time_ns
    LAST_RESULTS = res

    out_full = np.empty((B_FULL, C, OUT_H, OUT_W), np.float32)
    for s in range(B_LOC):
        for c in range(N_CORES):
            # [112, C, 2, 224] -> [C, 2, 112, 224] -> [C, 224, 224]
            arr = np.asarray(res.results[c]["out"][s]).astype(np.float32)
            out_full[assign[s][c]] = arr.transpose(1, 2, 0, 3).reshape(
                C, OUT_H, OUT_W)
    return out_full
